# revision 19
# baseline (speedup 1.0000x reference)
"""Trainium2 Bass kernel for nn_EquiConv (e3nn-style tensor product with
per-edge generated weights), data-parallel over edges on 8 NeuronCores.

Per 512-edge tile (4 blocks of 128 edges on partitions):
  PE : fwT = transpose(fw); hT = W1n^T @ fwT (fp32r); per block:
       w = hT-slice^T @ W2n' (fp32r) -> PSUM
  ACT: h = silu(hT); evacuate w PSUM -> SBUF bf16 (per block)
  DVE: tile-batched broadcast-muls + bf16 fold-trees (2x mode), per-edge
       factors via step-0 broadcast APs; all path constants folded into W2n'.

W2n' host prep: scaled by SILU_NORM/sqrt(64), per-path constants
(pw00, pw110/sqrt3, pw011/sqrt3, pw101/sqrt3) folded into the respective
column blocks, then permuted to w-major order per path:
  path00/110: col w*48+u   (u<32 -> w00, u>=32 -> w110)
  path011   : 1536 + w*32 + u
  path101   : 2048 + w*16 + u
"""
import math

import numpy as np

E_TOTAL = 65536
N_CORES = 8
E_CORE = E_TOTAL // N_CORES        # 8192
TILE_E = 1024
BLK = 128
NB = TILE_E // BLK                 # 4 blocks per tile
N_TILES = E_CORE // TILE_E         # 16
MUL0, MUL1 = 32, 16
FC_IN, FC_HID = 64, 64
WNUMEL = 2304
SILU_NORM = 1.6790
ISQRT3 = 1.0 / math.sqrt(3.0)
PW00 = math.sqrt(1.0 / (MUL0 * 2))
PW110I3 = math.sqrt(1.0 / (MUL1 * 2)) * ISQRT3
PW011I3 = math.sqrt(3.0 / (MUL0 * 2)) * ISQRT3
PW101I3 = math.sqrt(3.0 / (MUL1 * 2)) * ISQRT3

_NC_CACHE = {}


def _w2_prep(W2):
    """Scale + fold path constants + permute to w-major. Returns [64, 2304]."""
    W2n = W2.astype(np.float64) * (SILU_NORM / math.sqrt(FC_HID))
    W2n[:, 0:1024] *= PW00
    W2n[:, 1024:1536] *= PW110I3
    W2n[:, 1536:2048] *= PW011I3
    W2n[:, 2048:2304] *= PW101I3
    old = np.empty(WNUMEL, np.int64)
    for w in range(32):
        for u in range(48):
            old[w * 48 + u] = (u * 32 + w) if u < 32 else (1024 + (u - 32) * 32 + w)
    for w in range(16):
        for u in range(32):
            old[1536 + w * 32 + u] = 1536 + u * 16 + w
    for w in range(16):
        for u in range(16):
            old[2048 + w * 16 + u] = 2048 + u * 16 + w
    return np.ascontiguousarray(W2n[:, old]).astype(np.float32)


def _build():
    import concourse.tile as tile
    from concourse import bacc, mybir
    from concourse.masks import make_identity

    f32 = mybir.dt.float32
    f32r = mybir.dt.float32r
    bf16 = mybir.dt.bfloat16
    MULT = mybir.AluOpType.mult
    ADD = mybir.AluOpType.add
    AXX = mybir.AxisListType.X

    nc = bacc.Bacc("TRN2", target_bir_lowering=False, debug=False)
    fea_in1 = nc.declare_dram_parameter("fea_in1", [E_CORE, 80], f32, isOutput=False)
    fea_in2 = nc.declare_dram_parameter("fea_in2", [E_CORE, 4], f32, isOutput=False)
    fea_w = nc.declare_dram_parameter("fea_weight", [E_CORE, 64], f32, isOutput=False)
    W1n = nc.declare_dram_parameter("W1n", [64, 64], f32, isOutput=False)
    W2n = nc.declare_dram_parameter("W2n", [64, WNUMEL], f32, isOutput=False)
    out_d = nc.declare_dram_parameter("out", [E_CORE, 80], f32, isOutput=True)

    with tile.TileContext(nc) as tc, nc.allow_low_precision("bf16 contraction"):
        with (
            tc.tile_pool(name="consts", bufs=1) as consts,
            tc.tile_pool(name="ins", bufs=2) as insp,
            tc.tile_pool(name="mid", bufs=2) as mid,
            tc.tile_pool(name="wsb", bufs=2) as wsbp,
            tc.tile_pool(name="work", bufs=2) as work,
            tc.tile_pool(name="tmp", bufs=1) as tmpp,
            tc.tile_pool(name="outs", bufs=2) as outsp,
            tc.tile_pool(name="ps_w", bufs=2, space="PSUM") as ps_w,
            tc.tile_pool(name="ps_s", bufs=1, space="PSUM") as ps_s,
        ):
            ident = consts.tile([128, 128], f32)
            make_identity(nc, ident)
            w1_t = consts.tile([64, 64], f32r)
            nc.gpsimd.dma_start(w1_t[:], W1n[:])
            w2_t = consts.tile([64, WNUMEL], f32r)
            nc.gpsimd.dma_start(w2_t[:], W2n[:])

            for t in range(N_TILES):
                e0 = t * TILE_E
                # ---- batched input loads ----
                fwB = insp.tile([BLK, NB * 64], f32, tag="fwB")
                nc.sync.dma_start(
                    fwB[:].rearrange("p (b f) -> p b f", b=NB),
                    fea_w[e0:e0 + TILE_E].rearrange("(b p) f -> p b f", p=BLK))
                x1B = insp.tile([BLK, NB * 80], bf16, tag="x1B")
                nc.gpsimd.dma_start(
                    x1B[:].rearrange("p (b f) -> p b f", b=NB),
                    fea_in1[e0:e0 + TILE_E].rearrange("(b p) f -> p b f", p=BLK))
                x2B = insp.tile([BLK, NB * 4], bf16, tag="x2B")
                nc.gpsimd.dma_start(
                    x2B[:].rearrange("p (b f) -> p b f", b=NB),
                    fea_in2[e0:e0 + TILE_E].rearrange("(b p) f -> p b f", p=BLK))

                # views over the batched per-edge features
                x1v = x1B[:].rearrange("p (b f) -> p b f", b=NB)    # [128,4,80]
                x2v = x2B[:].rearrange("p (b f) -> p b f", b=NB)    # [128,4,4]

                # ---- fw transpose + mm1 + silu (per 512-half) ----
                fwT_sb = mid.tile([64, TILE_E], f32r, tag="fwT_sb")
                h_sb = mid.tile([64, TILE_E], f32r, tag="h_sb")
                for hf in range(TILE_E // 512):
                    fwT_ps = ps_s.tile([64, 512], f32, tag="fwT")
                    for b in range(4):
                        nc.tensor.transpose(
                            fwT_ps[:, b * BLK:(b + 1) * BLK],
                            fwB[:, (hf * 4 + b) * 64:(hf * 4 + b + 1) * 64],
                            ident[:])
                    nc.scalar.copy(
                        fwT_sb[:, hf * 512:(hf + 1) * 512], fwT_ps[:])
                    h_ps = ps_s.tile([64, 512], f32, tag="h")
                    nc.tensor.matmul(
                        h_ps[:], w1_t[:], fwT_sb[:, hf * 512:(hf + 1) * 512],
                        start=True, stop=True)
                    nc.scalar.activation(
                        h_sb[:, hf * 512:(hf + 1) * 512], h_ps[:],
                        mybir.ActivationFunctionType.Silu)

                # ---- mm2 + per-slice evacuation into batched w_sb ----
                w_sb = wsbp.tile([BLK, NB * WNUMEL], bf16, tag="w_sb")
                for b in range(NB):
                    lhs = h_sb[:, b * BLK:(b + 1) * BLK]
                    for half in range(2):
                        wp = ps_w.tile([BLK, 1024], f32, tag="wp")
                        for s in range(2):
                            nc.tensor.matmul(
                                wp[:, s * 512:(s + 1) * 512], lhs,
                                w2_t[:, half * 1024 + s * 512:
                                     half * 1024 + (s + 1) * 512],
                                start=True, stop=True)
                        nc.scalar.copy(
                            w_sb[:, b * WNUMEL + half * 1024:
                                 b * WNUMEL + (half + 1) * 1024], wp[:])
                    wq = ps_w.tile([BLK, 256], f32, tag="wq")
                    nc.tensor.matmul(
                        wq[:], lhs, w2_t[:, 2048:2304], start=True, stop=True)
                    nc.scalar.copy(
                        w_sb[:, b * WNUMEL + 2048:b * WNUMEL + 2304], wq[:])
                wv = w_sb[:].rearrange("p (b n) -> p b n", b=NB)    # [128,4,2304]

                # ---- per-edge features (tile-batched, constants in W2) ----
                # a0[b, 0:32] = x1_0 * x2_0 ; a0[b, 32+u] = sum_i x1_1[u,i]*x2_1[i]
                a0 = work.tile([BLK, NB * 48], bf16, tag="a0")
                a0v = a0[:].rearrange("p (b u) -> p b u", b=NB)
                nc.vector.tensor_tensor(
                    a0v[:, :, 0:32], x1v[:, :, 0:32],
                    x2v[:, :, 0:1].broadcast_to((BLK, NB, 32)), MULT)
                tbv = work.tile([BLK, NB * 48], bf16, tag="tbv")
                nc.vector.tensor_tensor(
                    tbv[:].rearrange("p (b u i) -> p b u i", b=NB, u=16),
                    x1v[:, :, 32:80].rearrange("p b (u i) -> p b u i", i=3),
                    x2v[:, :, 1:4].unsqueeze(2).broadcast_to((BLK, NB, 16, 3)),
                    MULT)
                nc.vector.tensor_reduce(
                    a0v[:, :, 32:48],
                    tbv[:].rearrange("p (b u i) -> p b u i", b=NB, u=16),
                    AXX, ADD)
                # x1sT[b, k, u] = x2_0 * x1_1[u, k]
                x1sT = work.tile([BLK, NB * 48], bf16, tag="x1sT")
                x1sTv = x1sT[:].rearrange("p (b k u) -> p b k u", b=NB, k=3)
                nc.vector.tensor_tensor(
                    x1sTv,
                    x1v[:, :, 32:80].rearrange("p b (u k) -> p b k u", k=3),
                    x2v[:, :, 0:1].unsqueeze(3).broadcast_to((BLK, NB, 3, 16)),
                    MULT)

                # ---- contraction: batched muls + bf16 fold trees ----
                outblk = outsp.tile([BLK, NB * 80], f32, tag="outblk")
                obv = outblk[:].rearrange("p (b f) -> p b f", b=NB)

                # path00/110: out0[b,w] = sum_u a0[b,u] * w[b, w*48+u]
                tmp00 = tmpp.tile([BLK, NB * 1536], bf16, tag="tmp00")
                t00 = tmp00[:].rearrange("p (b w u) -> p b w u", b=NB, w=32)
                nc.vector.tensor_tensor(
                    t00, wv[:, :, 0:1536].rearrange("p b (w u) -> p b w u", w=32),
                    a0v.unsqueeze(2).broadcast_to((BLK, NB, 32, 48)), MULT)
                f24 = tmpp.tile([BLK, NB * 768], bf16, tag="f24")
                v24 = f24[:].rearrange("p (b w u) -> p b w u", b=NB, w=32)
                nc.vector.tensor_tensor(v24, t00[:, :, :, 0:24],
                                        t00[:, :, :, 24:48], ADD)
                f12 = tmpp.tile([BLK, NB * 384], bf16, tag="f12")
                v12 = f12[:].rearrange("p (b w u) -> p b w u", b=NB, w=32)
                nc.vector.tensor_tensor(v12, v24[:, :, :, 0:12],
                                        v24[:, :, :, 12:24], ADD)
                f6 = tmpp.tile([BLK, NB * 192], bf16, tag="f6")
                v6 = f6[:].rearrange("p (b w u) -> p b w u", b=NB, w=32)
                nc.vector.tensor_tensor(v6, v12[:, :, :, 0:6],
                                        v12[:, :, :, 6:12], ADD)
                f3 = work.tile([BLK, NB * 96], bf16, tag="f3")
                v3 = f3[:].rearrange("p (b w u) -> p b w u", b=NB, w=32)
                nc.vector.tensor_tensor(v3, v6[:, :, :, 0:3],
                                        v6[:, :, :, 3:6], ADD)
                f1 = work.tile([BLK, NB * 32], f32, tag="f1")
                v1 = f1[:].rearrange("p (b w) -> p b w", b=NB)
                nc.vector.tensor_tensor(v1, v3[:, :, :, 0], v3[:, :, :, 1], ADD)
                nc.vector.tensor_tensor(obv[:, :, 0:32], v1, v3[:, :, :, 2], ADD)

                # path011: c[b,w] = sum_u x1_0[b,u] * w[b, 1536+w*32+u]
                tmp011 = tmpp.tile([BLK, NB * 512], bf16, tag="tmp011")
                t011 = tmp011[:].rearrange("p (b w u) -> p b w u", b=NB, w=16)
                nc.vector.tensor_tensor(
                    t011,
                    wv[:, :, 1536:2048].rearrange("p b (w u) -> p b w u", w=16),
                    x1v[:, :, 0:32].unsqueeze(2).broadcast_to((BLK, NB, 16, 32)),
                    MULT)
                g16 = tmpp.tile([BLK, NB * 256], bf16, tag="g16")
                u16 = g16[:].rearrange("p (b w u) -> p b w u", b=NB, w=16)
                nc.vector.tensor_tensor(u16, t011[:, :, :, 0:16],
                                        t011[:, :, :, 16:32], ADD)
                g8 = tmpp.tile([BLK, NB * 128], bf16, tag="g8")
                u8 = g8[:].rearrange("p (b w u) -> p b w u", b=NB, w=16)
                nc.vector.tensor_tensor(u8, u16[:, :, :, 0:8],
                                        u16[:, :, :, 8:16], ADD)
                g4 = tmpp.tile([BLK, NB * 64], bf16, tag="g4")
                u4 = g4[:].rearrange("p (b w u) -> p b w u", b=NB, w=16)
                nc.vector.tensor_tensor(u4, u8[:, :, :, 0:4], u8[:, :, :, 4:8], ADD)
                g2 = tmpp.tile([BLK, NB * 32], bf16, tag="g2")
                u2 = g2[:].rearrange("p (b w u) -> p b w u", b=NB, w=16)
                nc.vector.tensor_tensor(u2, u4[:, :, :, 0:2], u4[:, :, :, 2:4], ADD)
                cvec = work.tile([BLK, NB * 16], bf16, tag="cvec")
                cv = cvec[:].rearrange("p (b w) -> p b w", b=NB)
                nc.vector.tensor_tensor(cv, u2[:, :, :, 0], u2[:, :, :, 1], ADD)

                # path101: d[b,k,w] = sum_u x1sT[b,k,u] * w[b, 2048+w*16+u]
                tmp101 = tmpp.tile([BLK, NB * 3 * 256], bf16, tag="tmp101")
                t101 = tmp101[:].rearrange(
                    "p (b k w u) -> p b k w u", b=NB, k=3, w=16)
                nc.vector.tensor_tensor(
                    t101,
                    wv[:, :, 2048:2304].rearrange("p b (w u) -> p b w u", w=16)
                        .unsqueeze(2).broadcast_to((BLK, NB, 3, 16, 16)),
                    x1sTv.unsqueeze(3).broadcast_to((BLK, NB, 3, 16, 16)),
                    MULT)
                h8 = tmpp.tile([BLK, NB * 3 * 128], bf16, tag="h8")
                q8 = h8[:].rearrange("p (b k w u) -> p b k w u", b=NB, k=3, w=16)
                nc.vector.tensor_tensor(q8, t101[:, :, :, :, 0:8],
                                        t101[:, :, :, :, 8:16], ADD)
                h4 = tmpp.tile([BLK, NB * 3 * 64], bf16, tag="h4")
                q4 = h4[:].rearrange("p (b k w u) -> p b k w u", b=NB, k=3, w=16)
                nc.vector.tensor_tensor(q4, q8[:, :, :, :, 0:4],
                                        q8[:, :, :, :, 4:8], ADD)
                h2 = tmpp.tile([BLK, NB * 3 * 32], bf16, tag="h2")
                q2 = h2[:].rearrange("p (b k w u) -> p b k w u", b=NB, k=3, w=16)
                nc.vector.tensor_tensor(q2, q4[:, :, :, :, 0:2],
                                        q4[:, :, :, :, 2:4], ADD)
                dd = work.tile([BLK, NB * 48], bf16, tag="dd")
                ddv = dd[:].rearrange("p (b k w) -> p b k w", b=NB, k=3)
                nc.vector.tensor_tensor(ddv, q2[:, :, :, :, 0],
                                        q2[:, :, :, :, 1], ADD)

                # out1[b, w, k] = x2_1[b,k]*c[b,w] + d[b,k,w]
                tcx = work.tile([BLK, NB * 48], bf16, tag="tcx")
                tcv = tcx[:].rearrange("p (b k w) -> p b k w", b=NB, k=3)
                nc.vector.tensor_tensor(
                    tcv, cv.unsqueeze(2).broadcast_to((BLK, NB, 3, 16)),
                    x2v[:, :, 1:4].unsqueeze(3).broadcast_to((BLK, NB, 3, 16)),
                    MULT)
                nc.vector.tensor_tensor(
                    obv[:, :, 32:80].rearrange("p b (w k) -> p b k w", k=3),
                    tcv, ddv, ADD)

                nc.sync.dma_start(
                    out_d[e0:e0 + TILE_E].rearrange("(b p) f -> p b f", p=BLK),
                    obv)

    nc.finalize()
    return nc


def kernel(fea_in1, fea_in2, fea_weight, W1, W2):
    from concourse.bass_utils import run_bass_kernel_spmd

    if "nc" not in _NC_CACHE:
        _NC_CACHE["nc"] = _build()
    nc = _NC_CACHE["nc"]

    W1n = (W1 / math.sqrt(FC_IN)).astype(np.float32)
    W2n = _w2_prep(np.asarray(W2))
    fea_in1 = np.ascontiguousarray(fea_in1, dtype=np.float32)
    fea_in2 = np.ascontiguousarray(fea_in2, dtype=np.float32)
    fea_weight = np.ascontiguousarray(fea_weight, dtype=np.float32)

    in_maps = []
    for c in range(N_CORES):
        sl = slice(c * E_CORE, (c + 1) * E_CORE)
        in_maps.append({
            "fea_in1": fea_in1[sl],
            "fea_in2": fea_in2[sl],
            "fea_weight": fea_weight[sl],
            "W1n": W1n,
            "W2n": W2n,
        })
    res = run_bass_kernel_spmd(nc, in_maps, list(range(N_CORES)))
    return np.concatenate([res.results[c]["out"] for c in range(N_CORES)], axis=0)


# revision 20
# speedup vs baseline: 1.0070x; 1.0070x over previous
"""Trainium2 Bass kernel for nn_EquiConv (e3nn-style tensor product with
per-edge generated weights), data-parallel over edges on 8 NeuronCores.

Per 512-edge tile (4 blocks of 128 edges on partitions):
  PE : fwT = transpose(fw); hT = W1n^T @ fwT (fp32r); per block:
       w = hT-slice^T @ W2n' (fp32r) -> PSUM
  ACT: h = silu(hT); evacuate w PSUM -> SBUF bf16 (per block)
  DVE: tile-batched broadcast-muls + bf16 fold-trees (2x mode), per-edge
       factors via step-0 broadcast APs; all path constants folded into W2n'.

W2n' host prep: scaled by SILU_NORM/sqrt(64), per-path constants
(pw00, pw110/sqrt3, pw011/sqrt3, pw101/sqrt3) folded into the respective
column blocks, then permuted to w-major order per path:
  path00/110: col w*48+u   (u<32 -> w00, u>=32 -> w110)
  path011   : 1536 + w*32 + u
  path101   : 2048 + w*16 + u
"""
import math

import numpy as np

E_TOTAL = 65536
N_CORES = 8
E_CORE = E_TOTAL // N_CORES        # 8192
TILE_E = 1024
BLK = 128
NB = TILE_E // BLK                 # 4 blocks per tile
N_TILES = E_CORE // TILE_E         # 16
MUL0, MUL1 = 32, 16
FC_IN, FC_HID = 64, 64
WNUMEL = 2304
SILU_NORM = 1.6790
ISQRT3 = 1.0 / math.sqrt(3.0)
PW00 = math.sqrt(1.0 / (MUL0 * 2))
PW110I3 = math.sqrt(1.0 / (MUL1 * 2)) * ISQRT3
PW011I3 = math.sqrt(3.0 / (MUL0 * 2)) * ISQRT3
PW101I3 = math.sqrt(3.0 / (MUL1 * 2)) * ISQRT3

_NC_CACHE = {}


def _w2_prep(W2):
    """Scale + fold path constants + permute to w-major. Returns [64, 2304]."""
    W2n = W2.astype(np.float64) * (SILU_NORM / math.sqrt(FC_HID))
    W2n[:, 0:1024] *= PW00
    W2n[:, 1024:1536] *= PW110I3
    W2n[:, 1536:2048] *= PW011I3
    W2n[:, 2048:2304] *= PW101I3
    old = np.empty(WNUMEL, np.int64)
    for w in range(32):
        for u in range(48):
            old[w * 48 + u] = (u * 32 + w) if u < 32 else (1024 + (u - 32) * 32 + w)
    for w in range(16):
        for u in range(32):
            old[1536 + w * 32 + u] = 1536 + u * 16 + w
    for w in range(16):
        for u in range(16):
            old[2048 + w * 16 + u] = 2048 + u * 16 + w
    return np.ascontiguousarray(W2n[:, old]).astype(np.float32)


def _build():
    import concourse.tile as tile
    from concourse import bacc, mybir
    from concourse.masks import make_identity

    f32 = mybir.dt.float32
    f32r = mybir.dt.float32r
    bf16 = mybir.dt.bfloat16
    MULT = mybir.AluOpType.mult
    ADD = mybir.AluOpType.add
    AXX = mybir.AxisListType.X

    nc = bacc.Bacc("TRN2", target_bir_lowering=False, debug=False)
    fea_in1 = nc.declare_dram_parameter("fea_in1", [E_CORE, 80], f32, isOutput=False)
    fea_in2 = nc.declare_dram_parameter("fea_in2", [E_CORE, 4], f32, isOutput=False)
    fea_w = nc.declare_dram_parameter("fea_weight", [E_CORE, 64], f32, isOutput=False)
    W1n = nc.declare_dram_parameter("W1n", [64, 64], f32, isOutput=False)
    W2n = nc.declare_dram_parameter("W2n", [64, WNUMEL], f32, isOutput=False)
    out_d = nc.declare_dram_parameter("out", [E_CORE, 80], f32, isOutput=True)

    with tile.TileContext(nc) as tc, nc.allow_low_precision("bf16 contraction"):
        with (
            tc.tile_pool(name="consts", bufs=1) as consts,
            tc.tile_pool(name="ins", bufs=2) as insp,
            tc.tile_pool(name="mid", bufs=2) as mid,
            tc.tile_pool(name="wsb", bufs=2) as wsbp,
            tc.tile_pool(name="work", bufs=2) as work,
            tc.tile_pool(name="tmp", bufs=1) as tmpp,
            tc.tile_pool(name="outs", bufs=2) as outsp,
            tc.tile_pool(name="ps_w", bufs=2, space="PSUM") as ps_w,
            tc.tile_pool(name="ps_s", bufs=1, space="PSUM") as ps_s,
        ):
            ident = consts.tile([128, 128], f32)
            make_identity(nc, ident)
            w1_t = consts.tile([64, 64], f32r)
            nc.gpsimd.dma_start(w1_t[:], W1n[:])
            w2_t = consts.tile([64, WNUMEL], f32r)
            nc.gpsimd.dma_start(w2_t[:], W2n[:])

            for t in range(N_TILES):
                e0 = t * TILE_E
                # ---- batched input loads ----
                fwB = insp.tile([BLK, NB * 64], f32, tag="fwB")
                nc.sync.dma_start(
                    fwB[:].rearrange("p (b f) -> p b f", b=NB),
                    fea_w[e0:e0 + TILE_E].rearrange("(b p) f -> p b f", p=BLK))
                x1B = insp.tile([BLK, NB * 80], bf16, tag="x1B")
                nc.gpsimd.dma_start(
                    x1B[:].rearrange("p (b f) -> p b f", b=NB),
                    fea_in1[e0:e0 + TILE_E].rearrange("(b p) f -> p b f", p=BLK))
                x2B = insp.tile([BLK, NB * 4], bf16, tag="x2B")
                nc.gpsimd.dma_start(
                    x2B[:].rearrange("p (b f) -> p b f", b=NB),
                    fea_in2[e0:e0 + TILE_E].rearrange("(b p) f -> p b f", p=BLK))

                # views over the batched per-edge features
                x1v = x1B[:].rearrange("p (b f) -> p b f", b=NB)    # [128,4,80]
                x2v = x2B[:].rearrange("p (b f) -> p b f", b=NB)    # [128,4,4]

                # ---- fw transpose + mm1 + silu (per 512-half) ----
                fwT_sb = mid.tile([64, TILE_E], f32r, tag="fwT_sb")
                h_sb = mid.tile([64, TILE_E], f32r, tag="h_sb")
                for hf in range(TILE_E // 512):
                    fwT_ps = ps_s.tile([64, 512], f32, tag="fwT")
                    for b in range(4):
                        nc.tensor.transpose(
                            fwT_ps[:, b * BLK:(b + 1) * BLK],
                            fwB[:, (hf * 4 + b) * 64:(hf * 4 + b + 1) * 64],
                            ident[:])
                    nc.scalar.copy(
                        fwT_sb[:, hf * 512:(hf + 1) * 512], fwT_ps[:])
                    h_ps = ps_s.tile([64, 512], f32, tag="h")
                    nc.tensor.matmul(
                        h_ps[:], w1_t[:], fwT_sb[:, hf * 512:(hf + 1) * 512],
                        start=True, stop=True)
                    nc.scalar.activation(
                        h_sb[:, hf * 512:(hf + 1) * 512], h_ps[:],
                        mybir.ActivationFunctionType.Silu)

                # ---- mm2 + per-slice evacuation into batched w_sb ----
                w_sb = wsbp.tile([BLK, NB * WNUMEL], bf16, tag="w_sb")
                for b in range(NB):
                    lhs = h_sb[:, b * BLK:(b + 1) * BLK]
                    for half in range(2):
                        wp = ps_w.tile([BLK, 1024], f32, tag="wp")
                        for s in range(2):
                            nc.tensor.matmul(
                                wp[:, s * 512:(s + 1) * 512], lhs,
                                w2_t[:, half * 1024 + s * 512:
                                     half * 1024 + (s + 1) * 512],
                                start=True, stop=True)
                        nc.scalar.copy(
                            w_sb[:, b * WNUMEL + half * 1024:
                                 b * WNUMEL + (half + 1) * 1024], wp[:])
                    wq = ps_w.tile([BLK, 256], f32, tag="wq")
                    nc.tensor.matmul(
                        wq[:], lhs, w2_t[:, 2048:2304], start=True, stop=True)
                    nc.scalar.copy(
                        w_sb[:, b * WNUMEL + 2048:b * WNUMEL + 2304], wq[:])
                wv = w_sb[:].rearrange("p (b n) -> p b n", b=NB)    # [128,4,2304]

                # ---- per-edge features (tile-batched, constants in W2) ----
                # a0[b, 0:32] = x1_0 * x2_0 ; a0[b, 32+u] = sum_i x1_1[u,i]*x2_1[i]
                a0 = work.tile([BLK, NB * 48], bf16, tag="a0")
                a0v = a0[:].rearrange("p (b u) -> p b u", b=NB)
                nc.vector.tensor_tensor(
                    a0v[:, :, 0:32], x1v[:, :, 0:32],
                    x2v[:, :, 0:1].broadcast_to((BLK, NB, 32)), MULT)
                tbv = work.tile([BLK, NB * 48], bf16, tag="tbv")
                nc.vector.tensor_tensor(
                    tbv[:].rearrange("p (b u i) -> p b u i", b=NB, u=16),
                    x1v[:, :, 32:80].rearrange("p b (u i) -> p b u i", i=3),
                    x2v[:, :, 1:4].unsqueeze(2).broadcast_to((BLK, NB, 16, 3)),
                    MULT)
                nc.vector.tensor_reduce(
                    a0v[:, :, 32:48],
                    tbv[:].rearrange("p (b u i) -> p b u i", b=NB, u=16),
                    AXX, ADD)
                # x1sT[b, k, u] = x2_0 * x1_1[u, k]
                x1sT = work.tile([BLK, NB * 48], bf16, tag="x1sT")
                x1sTv = x1sT[:].rearrange("p (b k u) -> p b k u", b=NB, k=3)
                nc.vector.tensor_tensor(
                    x1sTv,
                    x1v[:, :, 32:80].rearrange("p b (u k) -> p b k u", k=3),
                    x2v[:, :, 0:1].unsqueeze(3).broadcast_to((BLK, NB, 3, 16)),
                    MULT)

                # ---- contraction: batched muls + bf16 fold trees ----
                outblk = outsp.tile([BLK, NB * 80], f32, tag="outblk")
                obv = outblk[:].rearrange("p (b f) -> p b f", b=NB)

                # path00/110: out0[b,w] = sum_u a0[b,u] * w[b, w*48+u]
                tmp00 = tmpp.tile([BLK, NB * 1536], bf16, tag="tmp00")
                t00 = tmp00[:].rearrange("p (b w u) -> p b w u", b=NB, w=32)
                nc.vector.tensor_tensor(
                    t00, wv[:, :, 0:1536].rearrange("p b (w u) -> p b w u", w=32),
                    a0v.unsqueeze(2).broadcast_to((BLK, NB, 32, 48)), MULT)
                f24 = tmpp.tile([BLK, NB * 768], bf16, tag="f24")
                v24 = f24[:].rearrange("p (b w u) -> p b w u", b=NB, w=32)
                nc.vector.tensor_tensor(v24, t00[:, :, :, 0:24],
                                        t00[:, :, :, 24:48], ADD)
                f12 = tmpp.tile([BLK, NB * 384], bf16, tag="f12")
                v12 = f12[:].rearrange("p (b w u) -> p b w u", b=NB, w=32)
                nc.vector.tensor_tensor(v12, v24[:, :, :, 0:12],
                                        v24[:, :, :, 12:24], ADD)
                f6 = tmpp.tile([BLK, NB * 192], bf16, tag="f6")
                v6 = f6[:].rearrange("p (b w u) -> p b w u", b=NB, w=32)
                nc.vector.tensor_tensor(v6, v12[:, :, :, 0:6],
                                        v12[:, :, :, 6:12], ADD)
                f3 = work.tile([BLK, NB * 96], bf16, tag="f3")
                v3 = f3[:].rearrange("p (b w u) -> p b w u", b=NB, w=32)
                nc.vector.tensor_tensor(v3, v6[:, :, :, 0:3],
                                        v6[:, :, :, 3:6], ADD)
                f1 = work.tile([BLK, NB * 32], f32, tag="f1")
                v1 = f1[:].rearrange("p (b w) -> p b w", b=NB)
                nc.vector.tensor_tensor(v1, v3[:, :, :, 0], v3[:, :, :, 1], ADD)
                nc.vector.tensor_tensor(obv[:, :, 0:32], v1, v3[:, :, :, 2], ADD)

                # path011: c[b,w] = sum_u x1_0[b,u] * w[b, 1536+w*32+u]
                tmp011 = tmpp.tile([BLK, NB * 512], bf16, tag="tmp011")
                t011 = tmp011[:].rearrange("p (b w u) -> p b w u", b=NB, w=16)
                nc.vector.tensor_tensor(
                    t011,
                    wv[:, :, 1536:2048].rearrange("p b (w u) -> p b w u", w=16),
                    x1v[:, :, 0:32].unsqueeze(2).broadcast_to((BLK, NB, 16, 32)),
                    MULT)
                g16 = tmpp.tile([BLK, NB * 256], bf16, tag="g16")
                u16 = g16[:].rearrange("p (b w u) -> p b w u", b=NB, w=16)
                nc.vector.tensor_tensor(u16, t011[:, :, :, 0:16],
                                        t011[:, :, :, 16:32], ADD)
                g8 = tmpp.tile([BLK, NB * 128], bf16, tag="g8")
                u8 = g8[:].rearrange("p (b w u) -> p b w u", b=NB, w=16)
                nc.vector.tensor_tensor(u8, u16[:, :, :, 0:8],
                                        u16[:, :, :, 8:16], ADD)
                g4 = tmpp.tile([BLK, NB * 64], bf16, tag="g4")
                u4 = g4[:].rearrange("p (b w u) -> p b w u", b=NB, w=16)
                nc.vector.tensor_tensor(u4, u8[:, :, :, 0:4], u8[:, :, :, 4:8], ADD)
                g2 = tmpp.tile([BLK, NB * 32], bf16, tag="g2")
                u2 = g2[:].rearrange("p (b w u) -> p b w u", b=NB, w=16)
                nc.vector.tensor_tensor(u2, u4[:, :, :, 0:2], u4[:, :, :, 2:4], ADD)
                cvec = work.tile([BLK, NB * 16], bf16, tag="cvec")
                cv = cvec[:].rearrange("p (b w) -> p b w", b=NB)
                nc.vector.tensor_tensor(cv, u2[:, :, :, 0], u2[:, :, :, 1], ADD)

                # path101: d[b,k,w] = sum_u x1sT[b,k,u] * w[b, 2048+w*16+u]
                tmp101 = tmpp.tile([BLK, NB * 3 * 256], bf16, tag="tmp101")
                t101 = tmp101[:].rearrange(
                    "p (b k w u) -> p b k w u", b=NB, k=3, w=16)
                nc.vector.tensor_tensor(
                    t101,
                    wv[:, :, 2048:2304].rearrange("p b (w u) -> p b w u", w=16)
                        .unsqueeze(2).broadcast_to((BLK, NB, 3, 16, 16)),
                    x1sTv.unsqueeze(3).broadcast_to((BLK, NB, 3, 16, 16)),
                    MULT)
                h8 = tmpp.tile([BLK, NB * 3 * 128], bf16, tag="h8")
                q8 = h8[:].rearrange("p (b k w u) -> p b k w u", b=NB, k=3, w=16)
                nc.vector.tensor_tensor(q8, t101[:, :, :, :, 0:8],
                                        t101[:, :, :, :, 8:16], ADD)
                h4 = tmpp.tile([BLK, NB * 3 * 64], bf16, tag="h4")
                q4 = h4[:].rearrange("p (b k w u) -> p b k w u", b=NB, k=3, w=16)
                nc.vector.tensor_tensor(q4, q8[:, :, :, :, 0:4],
                                        q8[:, :, :, :, 4:8], ADD)
                h2 = tmpp.tile([BLK, NB * 3 * 32], bf16, tag="h2")
                q2 = h2[:].rearrange("p (b k w u) -> p b k w u", b=NB, k=3, w=16)
                nc.vector.tensor_tensor(q2, q4[:, :, :, :, 0:2],
                                        q4[:, :, :, :, 2:4], ADD)
                dd = work.tile([BLK, NB * 48], bf16, tag="dd")
                ddv = dd[:].rearrange("p (b k w) -> p b k w", b=NB, k=3)
                nc.vector.tensor_tensor(ddv, q2[:, :, :, :, 0],
                                        q2[:, :, :, :, 1], ADD)

                # out1[b, w, k] = x2_1[b,k]*c[b,w] + d[b,k,w]
                tcx = work.tile([BLK, NB * 48], bf16, tag="tcx")
                tcv = tcx[:].rearrange("p (b k w) -> p b k w", b=NB, k=3)
                nc.vector.tensor_tensor(
                    tcv, cv.unsqueeze(2).broadcast_to((BLK, NB, 3, 16)),
                    x2v[:, :, 1:4].unsqueeze(3).broadcast_to((BLK, NB, 3, 16)),
                    MULT)
                nc.vector.tensor_tensor(
                    obv[:, :, 32:80].rearrange("p b (w k) -> p b k w", k=3),
                    tcv, ddv, ADD)

                nc.sync.dma_start(
                    out_d[e0:e0 + TILE_E].rearrange("(b p) f -> p b f", p=BLK),
                    obv)

    nc.finalize()
    return nc


def kernel(fea_in1, fea_in2, fea_weight, W1, W2):
    from concourse.bass_utils import run_bass_kernel_spmd

    if "nc" not in _NC_CACHE:
        _NC_CACHE["nc"] = _build()
    nc = _NC_CACHE["nc"]

    W1n = np.ascontiguousarray(np.asarray(W1, np.float32) / math.sqrt(FC_IN))
    W2n = _w2_prep(np.asarray(W2))
    fea_in1 = np.ascontiguousarray(np.asarray(fea_in1), dtype=np.float32)
    fea_in2 = np.ascontiguousarray(np.asarray(fea_in2), dtype=np.float32)
    fea_weight = np.ascontiguousarray(np.asarray(fea_weight), dtype=np.float32)

    in_maps = []
    for c in range(N_CORES):
        sl = slice(c * E_CORE, (c + 1) * E_CORE)
        in_maps.append({
            "fea_in1": fea_in1[sl],
            "fea_in2": fea_in2[sl],
            "fea_weight": fea_weight[sl],
            "W1n": W1n,
            "W2n": W2n,
        })
    res = run_bass_kernel_spmd(nc, in_maps, list(range(N_CORES)))
    return np.concatenate([res.results[c]["out"] for c in range(N_CORES)], axis=0)
